# revision 28
# baseline (speedup 1.0000x reference)
"""Trainium2 Bass kernel for nn_Block_74363063763569 (BEiT-style transformer block).

Data-parallel over batch across 8 NeuronCores (8 elems/core), zero collectives.

v2 design:
- Flat 1576-token stream per core (8 elems x 197 tokens), 13 token tiles of 128.
- fp8 e4m3 DoubleRow matmuls for qkv/v/proj/fc1/fc2 (2x PE throughput).
  Weights pre-scaled x64 (x512 with gamma folded for proj/fc2) to avoid e4m3
  subnormals; scale-backs folded into tensor_scalar / gelu scale / affine_then_add.
- Attention: per (elem, head): 2 full-128-partition logit MMs into one
  [128,2,197] PSUM, exp on scalar engine, exp(rpb) multiply on vector (fp8 out),
  1 fp8-DR attnxV with zero-padded V carrying a ones column for the softmax
  denominator; reciprocal via Ln+Exp(-x); partition_broadcast on gpsimd.
- v_bias folded into proj bias on host (softmax rows sum to 1).
"""
import sys, json
sys.path.insert(0, "/opt/trn_rl_repo")
import numpy as np


def _legalize_waits(bir_bytes, max_waits=1):
    """This container's walrus rejects >1 sync wait per instruction; split
    extras into preceding single-wait EventSemaphore instructions."""
    j = json.loads(bir_bytes)
    for f in j["functions"]:
        for b in f["blocks"]:
            out = []
            for inst in b["instructions"]:
                si = inst.get("sync_info")
                waits = si.get("on_wait", []) if si else []
                if len(waits) > max_waits:
                    keep, extra = waits[:max_waits], waits[max_waits:]
                    for k, w in enumerate(extra):
                        out.append({"debug": inst.get("debug", 0), "engine": inst["engine"],
                                    "ins": [], "name": f"{inst['name']}_w{k}",
                                    "opcode": "EventSemaphore", "outs": [],
                                    "sync_info": {"on_update": [], "on_wait": [w]}})
                    si["on_wait"] = keep
                out.append(inst)
            b["instructions"] = out
    return json.dumps(j).encode()


import concourse.bass as bass
import concourse.tile as tile
import concourse.mybir as mybir
from concourse.masks import make_identity

FP32 = mybir.dt.float32
BF16 = mybir.dt.bfloat16
F8 = mybir.dt.float8e4

B = 64
N = 197
C = 768
H = 12
D = 64
HID = 3072
NCORES = 8
BPC = B // NCORES           # 8 elems per core
TOK = BPC * N               # 1576 tokens per core
TOKP = 1664                 # padded to 13*128
NT = 13                     # token tiles (12x128 + 40)
LN_EPS = 1e-5
SWA = 64.0                  # weight prescale qkv/fc1
SWB = 512.0                 # weight prescale proj/fc2 (gamma folded)

TT = [(i * 128, 128 if i < 12 else TOK - 12 * 128) for i in range(NT)]
QCH = [(0, 256), (256, 394), (650, 394), (1044, 394), (1438, 138)]  # qk chunks, elem-aligned
MCH = [(0, 512), (512, 512), (1024, 512), (1536, 40)]  # mlp chunks (128-aligned)
CCH = [(0, 512), (512, 256)]                        # feature chunks for 768-wide out

AL = mybir.AluOpType
AF = mybir.ActivationFunctionType
DR = mybir.MatmulPerfMode.DoubleRow


def build_nc():
    nc = bass.Bass()

    x_d = nc.dram_tensor("x", [TOK, C], FP32, kind="ExternalInput")
    qkvT_d = nc.dram_tensor("qkvT8", [3, 128, 2, 3 * C], F8, kind="ExternalInput")
    projT_d = nc.dram_tensor("projT8", [3, 128, 2, C], F8, kind="ExternalInput")
    fc1T_d = nc.dram_tensor("fc1T8", [3, 128, 2, HID], F8, kind="ExternalInput")
    fc2T_d = nc.dram_tensor("fc2T8", [12, 128, 2, C], F8, kind="ExternalInput")
    erpb_d = nc.dram_tensor("exprpb", [128, H, 2, N], F8, kind="ExternalInput")
    qb_d = nc.dram_tensor("qbcol", [128, 6], FP32, kind="ExternalInput")
    kb_d = nc.dram_tensor("kbcol", [128, 6], FP32, kind="ExternalInput")
    f1b_d = nc.dram_tensor("f1bcol", [128, 24], FP32, kind="ExternalInput")
    pb_d = nc.dram_tensor("pb128", [128, C], FP32, kind="ExternalInput")
    f2b_d = nc.dram_tensor("f2brow", [C], BF16, kind="ExternalInput")
    y_d = nc.dram_tensor("y", [TOK, C], FP32, kind="ExternalOutput")

    with tile.TileContext(nc) as tc:
        with (
            tc.tile_pool(name="singles", bufs=1) as singles,
            tc.tile_pool(name="xf32", bufs=3) as xf32p,
            tc.tile_pool(name="xn8", bufs=2) as xn8p,
            tc.tile_pool(name="small", bufs=4) as small,
            tc.tile_pool(name="vt", bufs=2) as vtp,
            tc.tile_pool(name="expb", bufs=3) as expbp,
            tc.tile_pool(name="exp8", bufs=3) as exp8p,
            tc.tile_pool(name="rbc", bufs=2) as rbcp,
            tc.tile_pool(name="hT", bufs=2) as hTp,
            tc.tile_pool(name="out", bufs=2) as outp,
            tc.tile_pool(name="prj", bufs=2) as prjp,
            tc.tile_pool(name="ps_big", bufs=2, space="PSUM") as ps_big,
            tc.tile_pool(name="ps_fc2", bufs=3, space="PSUM") as ps_fc2,
            tc.tile_pool(name="ps_l", bufs=3, space="PSUM") as ps_lp,
        ):
            # ---- persistent weights / constants ----
            qkvT = [singles.tile([128, 2, 3 * C], F8, tag=f"qkvT{d}", name=f"qkvT{d}") for d in range(3)]
            projT = [singles.tile([128, 2, C], F8, tag=f"projT{d}", name=f"projT{d}") for d in range(3)]
            fc1T = [singles.tile([128, 2, HID], F8, tag=f"fc1T{d}", name=f"fc1T{d}") for d in range(3)]
            fc2T = [singles.tile([128, 2, C], F8, tag=f"fc2T{d}", name=f"fc2T{d}") for d in range(12)]
            erpb = singles.tile([128, H, 2, N], F8, tag="erpb")
            qb_sb = singles.tile([128, 6], FP32, tag="qb")
            kb_sb = singles.tile([128, 6], FP32, tag="kb")
            f1b_sb = singles.tile([128, 24], FP32, tag="f1b")
            pb128 = singles.tile([128, C], FP32, tag="pb128")
            f2b_sb = singles.tile([1, C], BF16, tag="f2b")
            identb = singles.tile([128, 128], BF16, tag="identb")
            ones_row = singles.tile([1, 128], BF16, tag="ones")
            eps_sb = singles.tile([128, 1], FP32, tag="eps")
            # all-ones (pad rows zeroed) DR weights for softmax denominators
            ones8 = singles.tile([128, 2, 32], F8, tag="ones8")

            # activations (persistent within the program)
            xnT = singles.tile([128, 6, TOKP], F8, tag="xnT")   # LN1 out T; reused for LN2
            qkT = [singles.tile([128, TOKP], F8, tag=f"qkT{ob}", name=f"qkT{ob}") for ob in range(12)]
            aT = [singles.tile([128, 2, TOKP], F8, tag=f"aT{d}", name=f"aT{d}") for d in range(3)]
            x0b = [singles.tile([128, C], BF16, tag=f"x0b{t}", name=f"x0b{t}") for t in range(NT)]
            x1s = [singles.tile([128, C], BF16, tag=f"x1_{t}", name=f"x1_{t}") for t in range(NT)]

            for d in range(3):
                nc.scalar.dma_start(qkvT[d][:], qkvT_d[d])
            nc.scalar.dma_start(qb_sb[:], qb_d[:])
            nc.scalar.dma_start(kb_sb[:], kb_d[:])
            nc.scalar.dma_start(erpb[:], erpb_d[:])
            nc.scalar.dma_start(pb128[:], pb_d[:])
            for d in range(3):
                nc.scalar.dma_start(projT[d][:], projT_d[d])
            for d in range(3):
                nc.gpsimd.dma_start(fc1T[d][:], fc1T_d[d])
            nc.gpsimd.dma_start(f1b_sb[:], f1b_d[:])
            for d in range(12):
                nc.gpsimd.dma_start(fc2T[d][:], fc2T_d[d])
            nc.gpsimd.dma_start(f2b_sb[:], f2b_d[None, :])
            make_identity(nc, identb[:])
            nc.vector.memset(ones_row[:], 1.0)
            nc.vector.memset(eps_sb[:], LN_EPS)
            nc.vector.memset(ones8[:, 0, :], 1.0)
            nc.vector.memset(ones8[:, 1, :], 0.0)
            nc.vector.memset(ones8[0:69, 1, :], 1.0)
            # zero the padded tail of k tiles (logits MM2 reads past 1576 for e=7)
            for ob in range(6, 12):
                nc.vector.memset(qkT[ob][:, TOK:TOKP], 0.0)

            # ---------------- LN + transpose helper ----------------
            def ln_tile(t, src_tiles, dst_T, phase):
                t0, tcnt = TT[t]
                xt = src_tiles[t]
                stats = small.tile([128, 3, 6], FP32, tag="st")
                mv = small.tile([128, 2], FP32, tag="mv")
                sd = small.tile([128, 1], FP32, tag="sd")
                rstd = small.tile([128, 1], FP32, tag="rs")
                for g in range(3):
                    nc.vector.bn_stats(stats[:tcnt, g, :], xt[:tcnt, g * 256:(g + 1) * 256])
                nc.vector.bn_aggr(mv[:tcnt], stats[:tcnt])
                nc.scalar.activation(sd[:tcnt], mv[:tcnt, 1:2], AF.Ln, bias=eps_sb[:tcnt])
                nc.scalar.activation(rstd[:tcnt], sd[:tcnt], AF.Exp, scale=-0.5)
                xn = xn8p.tile([128, C], BF16, tag="xn")
                if phase == 1:
                    # normalize on scalar: xn = xt * rstd + (-mean * rstd)
                    nmr = small.tile([128, 1], FP32, tag="nmr")
                    nc.vector.tensor_scalar(nmr[:tcnt], mv[:tcnt, 0:1], -1.0,
                                            rstd[:tcnt, 0:1], op0=AL.mult, op1=AL.mult)
                    nc.scalar.activation(xn[:tcnt, :], xt[:tcnt, :], AF.Identity,
                                         scale=rstd[:tcnt, 0:1], bias=nmr[:tcnt, 0:1])
                else:
                    nc.vector.tensor_scalar(
                        xn[:tcnt, :], xt[:tcnt, :], mv[:tcnt, 0:1], rstd[:tcnt, 0:1],
                        op0=AL.subtract, op1=AL.mult)
                for cb in range(6):
                    pt = ps_lp.tile([128, 128], BF16, tag="ps_l", name="pt")
                    nc.tensor.transpose(pt[:128, :tcnt], xn[:tcnt, cb * 128:(cb + 1) * 128],
                                        identb[:tcnt, :tcnt])
                    if phase == 2 or cb % 2 == 0:
                        nc.vector.tensor_copy(dst_T[:, cb, t0:t0 + tcnt], pt[:128, :tcnt])
                    else:
                        nc.scalar.copy(dst_T[:, cb, t0:t0 + tcnt], pt[:128, :tcnt])
                return xn

            # ---------------- P1: load x, LN1, transpose ----------------
            xf_tiles = {}
            def load_x(t):
                t0, tcnt = TT[t]
                xt = xf32p.tile([128, C], FP32, tag="xf")
                nc.sync.dma_start(xt[:tcnt, :], x_d[t0:t0 + tcnt, :])
                xf_tiles[t] = xt

            def ln1_tile(t):
                t0, tcnt = TT[t]
                xt = xf_tiles[t]
                ln_tile(t, xf_tiles, xnT, 1)
                nc.gpsimd.tensor_tensor(x0b[t][:tcnt, :], xt[:tcnt, :], pb128[:tcnt, :], op=AL.add)

            # ---------------- P2: q/k chunks ----------------
            def qk_chunk(ch):
                c0, csz = QCH[ch]
                for ob in range(12):
                    ps = ps_big.tile([128, 512], FP32, tag="big")
                    for d in range(3):
                        nc.tensor.matmul(
                            ps[:, :csz], qkvT[d][:, :, ob * 128:(ob + 1) * 128],
                            xnT[:, 2 * d:2 * d + 2, c0:c0 + csz],
                            start=(d == 0), stop=(d == 2), perf_mode=DR)
                    bias = qb_sb[:, ob:ob + 1] if ob < 6 else kb_sb[:, ob - 6:ob - 5]
                    if ob % 2 == 0:
                        nc.vector.tensor_scalar(
                            qkT[ob][:, c0:c0 + csz], ps[:, :csz], 1.0 / SWA, bias,
                            op0=AL.mult, op1=AL.add)
                    else:
                        nc.scalar.activation(
                            qkT[ob][:, c0:c0 + csz], ps[:, :csz], AF.Identity,
                            scale=1.0 / SWA, bias=bias)

            # ---------------- P3: v per elem (no ones col; denominator via ones8) ----------------
            def v_elem(e):
                eN = e * N
                vt = vtp.tile([128, 2, H, D], F8, tag="vt")
                nc.gpsimd.memset(vt[64:128, 1, :, :], 0.0)
                for j2, tcnt in ((0, 128), (1, 69)):
                    for (coff, csz) in CCH:
                        ps = ps_big.tile([128, 512], FP32, tag="big")
                        for d in range(3):
                            nc.tensor.matmul(
                                ps[:tcnt, :csz],
                                xnT[:, 2 * d:2 * d + 2, eN + j2 * 128: eN + j2 * 128 + tcnt],
                                qkvT[d][:, :, 2 * C + coff: 2 * C + coff + csz],
                                start=(d == 0), stop=(d == 2), perf_mode=DR)
                        h0 = coff // D
                        nh = csz // D
                        nc.vector.tensor_scalar(
                            vt[:tcnt, j2, h0:h0 + nh, 0:D], ps[:tcnt, :csz],
                            1.0 / SWA, None, op0=AL.mult)
                return vt

            # ---------------- P4: attention, head pairs, software pipelined ----------------
            def attn_pair_s1(e, k):
                # logits + exp + rpb-mult for heads (2k, 2k+1) into ONE fp8
                # tile [128, 2, 2N]: head h at free cols i*N..(i+1)*N
                eN = e * N
                e8 = exp8p.tile([128, 2, 2 * N], F8, tag="exp8")
                for i in range(2):
                    h = 2 * k + i
                    rh = (h % 2) * 64
                    kt = qkT[6 + h // 2]
                    qt = qkT[h // 2]
                    pl = ps_lp.tile([128, 2, N], FP32, tag="ps_l")
                    nc.tensor.matmul(pl[:, 0, :], kt[rh:rh + 64, eN:eN + 128],
                                     qt[rh:rh + 64, eN:eN + N])
                    nc.tensor.matmul(pl[:, 1, :], kt[rh:rh + 64, eN + 128:eN + 256],
                                     qt[rh:rh + 64, eN:eN + N])
                    eb = expbp.tile([128, 2, N], F8, tag="expb")
                    nc.scalar.activation(eb[:, :, :], pl[:, :, :], AF.Exp)
                    nc.vector.tensor_tensor(e8[:, :, i * N:(i + 1) * N], eb[:, :, :],
                                            erpb[:, h, :, :], op=AL.mult)
                return e8

            _cpeng = [0]
            def attn_pair_s2(e, k, vt, e8):
                # attn x V for both heads into one PSUM tile (partition 0, the
                # two heads at different free offsets — DR dst must start at
                # partition 0); ONE masked-ones DR matmul gives both
                # denominators; one Ln + one Exp + one K=1 broadcast-MM + one
                # copy; 2 norms.
                eN = e * N
                h0, h1 = 2 * k, 2 * k + 1
                po = ps_fc2.tile([64, 2, 256], FP32, tag="fc2", name="po")
                nc.tensor.matmul(po[0:D, 0, 0:N], vt[:, :, h0, :], e8[:, :, 0:N],
                                 perf_mode=DR)
                nc.tensor.matmul(po[0:D, 1, 0:N], vt[:, :, h1, :], e8[:, :, N:2 * N],
                                 perf_mode=DR)
                pd = ps_fc2.tile([32, 512], FP32, tag="fc2", name="pd")
                nc.tensor.matmul(pd[0:32, 0:2 * N], ones8[:, :, :], e8[:, :, :],
                                 perf_mode=DR)
                lden = small.tile([1, 2 * N], FP32, tag="lden")
                rr = small.tile([1, 2 * N], BF16, tag="rr")
                nc.scalar.activation(lden[:, :], pd[0:1, 0:2 * N], AF.Ln)
                nc.scalar.activation(rr[:, :], lden[:, :], AF.Exp, scale=-1.0)
                pdn = ps_fc2.tile([64, 512], FP32, tag="fc2", name="pdn")
                nc.tensor.matmul(pdn[0:D, 0:2 * N], ones_row[0:1, 0:D], rr[0:1, :])
                db = rbcp.tile([64, 2 * N], BF16, tag="rbc")
                if _cpeng[0] % 2 == 0:
                    nc.vector.tensor_copy(db[:, :], pdn[0:D, 0:2 * N])
                else:
                    nc.scalar.copy(db[:, :], pdn[0:D, 0:2 * N])
                _cpeng[0] += 1
                nc.vector.tensor_tensor(
                    aT[k // 2][0:D, k % 2, eN:eN + N],
                    po[0:D, 0, 0:N], db[:, 0:N], op=AL.mult)
                nc.vector.tensor_tensor(
                    aT[k // 2][D:2 * D, k % 2, eN:eN + N],
                    po[0:D, 1, 0:N], db[:, N:2 * N], op=AL.mult)

            # ---------------- P5: proj + residual per token tile ----------------
            def proj_tile(t):
                t0, tcnt = TT[t]
                for (coff, csz) in CCH:
                    ps = ps_big.tile([128, 512], FP32, tag="big")
                    for d in range(3):
                        nc.tensor.matmul(
                            ps[:tcnt, :csz], aT[d][:, :, t0:t0 + tcnt],
                            projT[d][:, :, coff:coff + csz],
                            start=(d == 0), stop=(d == 2), perf_mode=DR)
                    pt = prjp.tile([128, 512], BF16, tag="prj")
                    if coff == 0:
                        nc.scalar.activation(pt[:tcnt, :csz], ps[:tcnt, :csz], AF.Copy,
                                             scale=1.0 / SWB)
                    else:
                        nc.vector.tensor_scalar(pt[:tcnt, :csz], ps[:tcnt, :csz],
                                                1.0 / SWB, None, op0=AL.mult)
                    nc.vector.tensor_tensor(
                        x1s[t][:tcnt, coff:coff + csz], pt[:tcnt, :csz],
                        x0b[t][:tcnt, coff:coff + csz], op=AL.add)

            # ---------------- P6: LN2 ----------------
            x1_map = {t: x1s[t] for t in range(NT)}
            def ln2_tile(t):
                ln_tile(t, x1_map, xnT, 2)

            # ---------------- P7: MLP ----------------
            hT_tiles = {}
            def fc1_chunk(c):
                c0, csz = MCH[c]
                ht = hTp.tile([128, 24, 512], F8, tag="hT")
                for ob in range(24):
                    ps = ps_big.tile([128, 512], FP32, tag="big")
                    for d in range(3):
                        nc.tensor.matmul(
                            ps[:, :csz], fc1T[d][:, :, ob * 128:(ob + 1) * 128],
                            xnT[:, 2 * d:2 * d + 2, c0:c0 + csz],
                            start=(d == 0), stop=(d == 2), perf_mode=DR)
                    nc.scalar.activation(ht[:, ob, :csz], ps[:, :csz], AF.Gelu,
                                         bias=f1b_sb[:, ob:ob + 1], scale=1.0 / SWA)
                hT_tiles[c] = ht

            def fc2_chunk(c):
                c0, csz = MCH[c]
                ht = hT_tiles[c]
                nsub = (csz + 127) // 128
                for k in range(nsub):
                    tk0 = k * 128
                    tcnt = min(128, csz - tk0)
                    t = 4 * c + k
                    ot = outp.tile([128, C], FP32, tag="out")
                    for (coff, cw) in CCH:
                        ps = ps_fc2.tile([128, 512], FP32, tag="fc2")
                        for d in range(12):
                            nc.tensor.matmul(
                                ps[:tcnt, :cw], ht[:, 2 * d:2 * d + 2, tk0:tk0 + tcnt],
                                fc2T[d][:, :, coff:coff + cw],
                                start=(d == 0), stop=(d == 11), perf_mode=DR)
                        nc.vector.tensor_scalar(ps[:tcnt, :cw], ps[:tcnt, :cw],
                                                1.0 / SWB, None, op0=AL.mult)
                        nc.vector.tensor_tensor(
                            ot[:tcnt, coff:coff + cw], ps[:tcnt, :cw],
                            x1s[t][:tcnt, coff:coff + cw], op=AL.add)
                    gt0 = c0 + tk0
                    nc.gpsimd.dma_start(y_d[gt0:gt0 + tcnt, :], ot[:tcnt, :])

            # ================= issue order =================
            # attention pair pipeline, lookahead-2: stage1 (logits+exp+mult) of
            # pairs i+1, i+2 issue before stage2 (attnV+denoms+norm) of pair i;
            # attention for elem e interleaves right behind the qk chunk that
            # completes its columns, overlapping LN1/qk vector work with PE
            vts = {}
            pend = []
            state = {"proj_done": 0}

            def flush_pair():
                e0, k0, e80 = pend.pop(0)
                attn_pair_s2(e0, k0, vts[e0], e80)

            def issue_attn(e):
                for k in range(H // 2):
                    if k == 0:
                        vts[e] = v_elem(e)
                    e8 = attn_pair_s1(e, k)
                    pend.append((e, k, e8))
                    if len(pend) > 2:
                        flush_pair()
                ready_tok = e * N  # elems < e fully flushed -> aT final
                while (state["proj_done"] < NT and
                       TT[state["proj_done"]][0] + TT[state["proj_done"]][1] <= ready_tok):
                    proj_tile(state["proj_done"])
                    state["proj_done"] += 1

            for t in range(3):
                load_x(t)
            for t in range(2):
                ln1_tile(t)
            qk_chunk(0)           # cols 0:256 -> elem 0 ready
            issue_attn(0)
            for t in range(3, 6):
                load_x(t)
            for t in range(2, 6):
                ln1_tile(t)
            qk_chunk(1)           # -> cols 0:650, elems 1-2
            issue_attn(1)
            issue_attn(2)
            for t in range(6, 9):
                load_x(t)
                ln1_tile(t)
            qk_chunk(2)           # -> cols 0:1044, elems 3-4
            issue_attn(3)
            # tiles 0-3 final (ready_tok 591 >= 512): start MLP chunk 0 inside
            # the attention phase to keep the PE dense
            for t in range(4):
                ln2_tile(t)
            fc1_chunk(0)
            fc2_chunk(0)
            issue_attn(4)
            for t in range(9, 12):
                load_x(t)
                ln1_tile(t)
            qk_chunk(3)           # -> cols 0:1438, elems 5-6
            issue_attn(5)
            for t in range(4, 7):
                ln2_tile(t)
            issue_attn(6)
            for t in range(12, 13):
                load_x(t)
                ln1_tile(t)
            qk_chunk(4)           # -> full, elem 7
            for t in range(7, 8):
                ln2_tile(t)
            fc1_chunk(1)
            fc2_chunk(1)
            issue_attn(7)
            while pend:
                flush_pair()
            while state["proj_done"] < NT:
                proj_tile(state["proj_done"])
                state["proj_done"] += 1
            for t in range(8, 13):
                ln2_tile(t)
            fc1_chunk(2)
            fc2_chunk(2)
            fc1_chunk(3)
            fc2_chunk(3)

    return nc


def fold_weights(inputs):
    """Host-side folding. Returns dict of per-core-shared input arrays."""
    import ml_dtypes
    f32 = np.float32
    bf16 = ml_dtypes.bfloat16
    f8 = ml_dtypes.float8_e4m3
    g = {k: np.asarray(v) for k, v in inputs.items()}
    n1w, n1b = g["n1_w"].astype(f32), g["n1_b"].astype(f32)
    n2w, n2b = g["n2_w"].astype(f32), g["n2_b"].astype(f32)
    g1, g2 = g["gamma1"].astype(f32), g["gamma2"].astype(f32)
    qkv_w = g["qkv_w"].astype(f32)
    q_bias, v_bias = g["q_bias"].astype(f32), g["v_bias"].astype(f32)
    proj_w, proj_b = g["proj_w"].astype(f32), g["proj_b"].astype(f32)
    fc1_w, fc1_b = g["fc1_w"].astype(f32), g["fc1_b"].astype(f32)
    fc2_w, fc2_b = g["fc2_w"].astype(f32), g["fc2_b"].astype(f32)

    qkv_bias = np.concatenate([q_bias, np.zeros_like(q_bias), v_bias])
    Wq = qkv_w * n1w[None, :]
    bq = qkv_bias + qkv_w @ n1b
    scale = D ** -0.5
    Wq[:C] *= scale
    bq[:C] *= scale

    def pack_dr(WT, sw):
        # WT [K, O] fp32 -> [K/256, 128, 2, O] fp8 with k-subtile pairs on dim2
        K, O = WT.shape
        a = (sw * WT).reshape(K // 128, 128, O)
        return np.ascontiguousarray(np.stack([a[0::2], a[1::2]], axis=2)).astype(f8)

    qkvT8 = pack_dr(np.ascontiguousarray(Wq.T), SWA)                     # [3,128,2,2304]
    projT8 = pack_dr(np.ascontiguousarray((g1[:, None] * proj_w).T), SWB)
    fc1T8 = pack_dr(np.ascontiguousarray((fc1_w * n2w[None, :]).T), SWA)
    fc2T8 = pack_dr(np.ascontiguousarray((g2[:, None] * fc2_w).T), SWB)

    f1b = fc1_b + fc1_w @ n2b
    vb = bq[2 * C:]
    pb_eff = g1 * (proj_b + proj_w @ vb)

    table = g["rel_bias_table"].astype(f32)
    idx = np.asarray(g["rel_index"]).reshape(-1)
    rpb = table[idx].reshape(N, N, H).transpose(2, 0, 1)   # [h, tq, tk]
    rpbT = rpb.transpose(0, 2, 1)                          # [h, tk, tq]
    erpb = np.ones((128, H, 2, N), np.float32)
    for h in range(H):
        erpb[:, h, 0, :] = np.exp(rpbT[h][0:128, :])
        erpb[0:69, h, 1, :] = np.exp(rpbT[h][128:197, :])

    col = lambda v, k: np.ascontiguousarray(v.reshape(k, 128).T)

    return {
        "qkvT8": qkvT8,
        "projT8": projT8,
        "fc1T8": fc1T8,
        "fc2T8": fc2T8,
        "exprpb": erpb.astype(f8),
        "qbcol": col(bq[:C], 6),
        "kbcol": col(bq[C:2 * C], 6),
        "f1bcol": col(f1b, 24),
        "pb128": np.ascontiguousarray(np.broadcast_to(pb_eff[None, :], (128, C))),
        "f2brow": (SWB * g2 * fc2_b).astype(bf16),
    }


_CACHE = {}


def _get_nc():
    if "nc" not in _CACHE:
        nc = build_nc()
        patched = _legalize_waits(nc.to_json_bytes())
        nc.to_json_bytes = lambda: patched
        _CACHE["nc"] = nc
    return _CACHE["nc"]


def kernel(**inputs):
    from concourse.bass_utils import run_bass_kernel_spmd
    nc = _get_nc()
    folded = fold_weights(inputs)
    x = np.ascontiguousarray(np.asarray(inputs["x"], dtype=np.float32))
    assert x.shape == (B, N, C), x.shape
    in_maps = []
    for c in range(NCORES):
        m = dict(folded)
        m["x"] = np.ascontiguousarray(
            x[c * BPC:(c + 1) * BPC].reshape(TOK, C))
        in_maps.append(m)
    res = run_bass_kernel_spmd(nc, in_maps, core_ids=list(range(NCORES)))
    out = np.concatenate(
        [res.results[c]["y"].reshape(BPC, N, C) for c in range(NCORES)], axis=0)
    return out.astype(np.float32)
